# revision 4
# baseline (speedup 1.0000x reference)
"""Trainium2 Bass kernel for GQA attention with RoPE (dense transformer).

Problem: B=2, S=2048, H=2048, 16 query heads / 4 KV heads, head_dim 128,
causal flash-style attention, fused QKV + o_proj.

Sharding (8 cores): (batch, head-group) grid. Core c handles batch c//4 and
head group c%4 (4 query heads + their shared KV head). o_proj is computed as
per-group partials reduced on host (tensor-parallel o_proj input split).

v2 vs baseline (336us):
  - bf16 activations/weights end-to-end (PE rate is identical to f32r, but
    DMA bytes and DVE element throughput both improve 2x; accuracy measured
    ~2e-3 rel, threshold 2e-2). PSUM accumulation stays fp32.
  - Softmax denominators: per (h,chunk) the exp tiles are accumulated over
    k-tiles on the DVE, then ONE ones-matmul computes the partition sum
    (was: one ones-matmul per k-tile; -144 PE matmuls).
  - Batched DMA: x loads as 1MB half-chunks, weights as whole tensors,
    outputs as one 0.5MB DMA per 128-token tile. Weights go on the ACT
    HWDGE queue, x/out on the SP queue.
  - o_proj is emitted interleaved (right after each q-chunk's attention),
    not as a tail phase, so its matmuls fill PE gaps left by exp pacing
    and the output DMA is spread across the kernel.

On-core layout: activations live as [feature, token] ("transposed") so the
feature contraction dims land on SBUF partitions for the PE array.
Causal masking: fully-masked k-tiles are skipped entirely; diagonal tiles
get a zero-fill triangle (affine_select on GpSimd) after exp.
"""
import math

import numpy as np

import concourse.bass as bass
import concourse.mybir as mybir
import concourse.tile as tile
from concourse import bacc
from concourse.bass_utils import run_bass_kernel_spmd
from concourse.masks import make_identity

B, S, H = 2, 2048, 2048
NH, KVH, HD = 16, 4, 128
G = 4                 # head groups (= KVH); grid = G x B = 8 cores
GQ = NH // KVH        # query heads per group
QD = GQ * HD          # per-core q dim (512)
KC = H // 128         # contraction chunks for projections (16)
TC = 4                # token chunks of 512
TT = S // 128         # 128-token tiles (16)

F32 = mybir.dt.float32
BF = mybir.dt.bfloat16
AF = mybir.ActivationFunctionType

_NC = None


def _emit(nc):
    xT = nc.dram_tensor("xT", [H, S], BF, kind="ExternalInput").ap()
    wqT = nc.dram_tensor("wqT", [H, QD], BF, kind="ExternalInput").ap()
    wkT = nc.dram_tensor("wkT", [H, HD], BF, kind="ExternalInput").ap()
    wvT = nc.dram_tensor("wvT", [H, HD], BF, kind="ExternalInput").ap()
    woT = nc.dram_tensor("woT", [QD, H], BF, kind="ExternalInput").ap()
    cosT = nc.dram_tensor("cosT", [HD, S], BF, kind="ExternalInput").ap()
    sinS = nc.dram_tensor("sinS", [HD, S], BF, kind="ExternalInput").ap()
    bqkv = nc.dram_tensor("bqkv", [128, 6], F32, kind="ExternalInput").ap()
    onesd = nc.dram_tensor("onesd", [128, 128], BF, kind="ExternalInput").ap()
    outp = nc.dram_tensor("outp", [S, H], BF, kind="ExternalOutput").ap()

    xT3 = xT.rearrange("(ko p) t -> p ko t", p=128)
    wqT3 = wqT.rearrange("(ko p) m -> p ko m", p=128)
    wkT3 = wkT.rearrange("(ko p) m -> p ko m", p=128)
    wvT3 = wvT.rearrange("(ko p) m -> p ko m", p=128)
    woT3 = woT.rearrange("(ic p) o -> p ic o", p=128)

    with tile.TileContext(nc) as tc:
        with (
            tc.tile_pool(name="persist", bufs=1) as pp,
            tc.tile_pool(name="qfp", bufs=2) as pqf,
            tc.tile_pool(name="cd", bufs=1) as pd,
            tc.tile_pool(name="expp", bufs=1) as pe,
            tc.tile_pool(name="psum8", bufs=1, space="PSUM") as ps8,
        ):
            # persistent per-chunk K/V (split per t-chunk to keep dep ranges
            # disjoint between the producing chunk and attention readers)
            kf = [pp.tile([128, 512], BF, name=f"kf{t}") for t in range(TC)]
            v_sb = [pp.tile([128, 4, HD], BF, name=f"vsb{t}")
                    for t in range(TC)]
            ofl = pd.tile([128, GQ, S], BF)       # normalized attn outT

            # ---- constants ----
            bias_sb = pp.tile([128, 6], F32)
            nc.gpsimd.dma_start(bias_sb[:, :], bqkv)
            ident = pp.tile([128, 128], BF)
            make_identity(nc, ident[:, :])
            ones_mat = pp.tile([128, 128], BF)
            nc.gpsimd.dma_start(ones_mat[:, :], onesd)

            def jspan(qc, j):
                if j < 4 * qc:
                    q0, n = 512 * qc, 512
                else:
                    q0 = 128 * j
                    n = 512 * (qc + 1) - q0
                return q0, n, q0 - 512 * qc

            def attention(qc, qf_t):
                """flash attention for q-chunk qc over k-tiles 0..4qc+3"""
                qs = slice(512 * qc, 512 * qc + 512)
                nj = 4 * qc + 4
                for h in range(GQ):
                    exs = pe.tile([128, 512], BF, tag="exs", bufs=3,
                                  name=f"exs_{h}_{qc}")
                    exts = []
                    for j in range(nj):
                        q0, n, off = jspan(qc, j)
                        ql = q0 - 512 * qc
                        ps = ps8.tile([128, 512], F32, tag=f"A{j % 4}",
                                      name=f"ps_{h}_{qc}_{j}")
                        nc.tensor.matmul(
                            ps[:, 0:n], kf[j // 4][:, 128 * (j % 4):
                                                   128 * (j % 4) + 128],
                            qf_t[:, h, ql:ql + n], start=True, stop=True)
                        ex = pe.tile([128, 512], BF, tag="E", bufs=20,
                                     name=f"ex_{h}_{qc}_{j}")
                        nc.scalar.activation(ex[:, 0:n], ps[:, 0:n], AF.Exp)
                        if j >= 4 * qc:
                            # zero the strictly-lower (q < k) triangle
                            nc.gpsimd.affine_select(
                                out=ex[:, 0:128], in_=ex[:, 0:128],
                                compare_op=mybir.AluOpType.is_ge, fill=0.0,
                                base=0, pattern=[[1, 128]],
                                channel_multiplier=-1)
                        if j == 0:
                            nc.vector.tensor_copy(exs[:, :], ex[:, :])
                        else:
                            nc.vector.tensor_add(exs[:, ql:ql + n],
                                                 exs[:, ql:ql + n],
                                                 ex[:, 0:n])
                        exts.append(ex)
                    p_sum = ps8.tile([128, 512], F32, tag="Bt", bufs=2,
                                     name=f"psum_{h}_{qc}")
                    nc.tensor.matmul(p_sum[:, :], ones_mat[:, :], exs[:, :],
                                     start=True, stop=True)
                    bc = pe.tile([128, 512], F32, tag="bc", bufs=2,
                                 name=f"bc_{h}_{qc}")
                    nc.vector.reciprocal_approx_fast(bc[:, :], p_sum[:, :])
                    p_o = ps8.tile([128, 512], F32, tag="Ct", bufs=2,
                                   name=f"po_{h}_{qc}")
                    for j in range(nj):
                        q0, n, off = jspan(qc, j)
                        nc.tensor.matmul(
                            p_o[:, off:off + n],
                            v_sb[j // 4][:, j % 4, :],
                            exts[j][:, 0:n], start=(j == 0), stop=(j == nj - 1))
                    nc.vector.tensor_mul(ofl[:, h, qs], p_o[:, :], bc[:, :])

            def oproj(qc, pwo, wo_sb):
                """o_proj partials for the 4 token-tiles of q-chunk qc."""
                for tt in range(4 * qc, 4 * qc + 4):
                    tsl = slice(128 * tt, 128 * tt + 128)
                    tags = ["Bt", "Bt", "Ct", "Ct"]
                    pfs = [ps8.tile([128, 512], F32, tag=tags[oc], bufs=2,
                                    name=f"pf_{tt}_{oc}")
                           for oc in range(4)]
                    for ic in range(GQ):
                        for oc in range(4):
                            osl = slice(512 * oc, 512 * oc + 512)
                            nc.tensor.matmul(
                                pfs[oc][:, :], ofl[:, ic, tsl],
                                wo_sb[:, ic, osl],
                                start=(ic == 0), stop=(ic == GQ - 1))
                    fo = pwo.tile([128, 4, 512], BF, tag="fo", bufs=3,
                                  name=f"fo_{tt}")
                    for oc in range(4):
                        nc.scalar.copy(fo[:, oc, :], pfs[oc][:, :])
                    nc.sync.dma_start(outp[tsl, :], fo[:, :, :])

            # ============ interleaved projections + attention =============
            qf_tiles = [None] * TC
            with (
                tc.tile_pool(name="projw", bufs=1) as pw,
                tc.tile_pool(name="projx", bufs=1) as px,
                tc.tile_pool(name="rope", bufs=1) as pr,
                tc.tile_pool(name="wop", bufs=1) as pwo,
            ):
                wq_sb = pw.tile([128, KC, QD], BF)
                wk_sb = pw.tile([128, KC, HD], BF)
                wv_sb = pw.tile([128, KC, HD], BF)
                cos_sb = pw.tile([128, S], BF)
                sin_sb = pw.tile([128, S], BF)
                wo_sb = pwo.tile([128, GQ, H], BF)

                # chunk-0 inputs: x halves on the SP queue, weights on the
                # ACT queue, ordered so the first accumulation can start
                # after ~1MB of each has landed.
                xh = [[None, None] for _ in range(TC)]

                def dma_x(t):
                    for half in range(2):
                        xt = px.tile([128, 8, 512], BF, tag="xh", bufs=4,
                                     name=f"xh_{t}_{half}")
                        nc.sync.dma_start(
                            xt[:, :, :],
                            xT3[:, 8 * half:8 * half + 8,
                                512 * t:512 * t + 512])
                        xh[t][half] = xt

                dma_x(0)
                nc.scalar.dma_start(wq_sb[:, 0:8, :], wqT3[:, 0:8, :])
                nc.scalar.dma_start(wk_sb[:, :, :], wkT3)
                nc.scalar.dma_start(wv_sb[:, :, :], wvT3)
                nc.scalar.dma_start(wq_sb[:, 8:16, :], wqT3[:, 8:16, :])
                nc.scalar.dma_start(cos_sb[:, :], cosT)
                nc.scalar.dma_start(sin_sb[:, :], sinS)

                for t in range(TC):
                    ts = slice(512 * t, 512 * t + 512)
                    if t + 1 < TC:
                        dma_x(t + 1)
                    if t == 0:
                        nc.scalar.dma_start(wo_sb[:, :, :], woT3)

                    pq = [ps8.tile([128, 512], F32, tag=f"A{m}",
                                   name=f"pq{m}_{t}")
                          for m in range(GQ)]
                    pk = ps8.tile([128, 512], F32, tag="Bt", bufs=2,
                                  name=f"pk_{t}")
                    pv = ps8.tile([128, 512], F32, tag="Bt", bufs=2,
                                  name=f"pv_{t}")
                    for ko in range(KC):
                        st = (ko == 0)
                        sp = (ko == KC - 1)
                        xc = xh[t][ko // 8][:, ko % 8, :]
                        for m in range(GQ):
                            nc.tensor.matmul(
                                pq[m][:, :],
                                wq_sb[:, ko, 128 * m:128 * m + 128],
                                xc, start=st, stop=sp)
                        nc.tensor.matmul(pk[:, :], wk_sb[:, ko, :],
                                         xc, start=st, stop=sp)
                        nc.tensor.matmul(pv[:, :], wv_sb[:, ko, :],
                                         xc, start=st, stop=sp)

                    # v: evict with bias, then transpose to natural layout
                    vT_t = pr.tile([128, 512], BF, tag="vT", bufs=2,
                                   name=f"vT_{t}")
                    nc.scalar.activation(vT_t[:, :], pv[:, :], AF.Identity,
                                         bias=bias_sb[:, 5:6])
                    for st4 in range(4):
                        ptr = ps8.tile([128, 128], BF, tag="Ct", bufs=2,
                                       name=f"ptr_{t}_{st4}")
                        nc.tensor.transpose(
                            ptr[:, :], vT_t[:, 128 * st4:128 * st4 + 128],
                            ident[:, :])
                        nc.scalar.copy(v_sb[t][:, st4, :], ptr[:, :])

                    # evict + bias; RoPE for q/k on DVE in [d, tok] layout
                    qf_t = pqf.tile([128, GQ, 512], BF, tag="qf",
                                    name=f"qf_{t}")
                    qf_tiles[t] = qf_t
                    for m in [GQ, 0, 1, 2, 3]:
                        raw = pr.tile([128, 512], BF, tag="raw", bufs=3,
                                      name=f"raw_{t}_{m}")
                        src_ps = pq[m][:, :] if m < GQ else pk[:, :]
                        bcol = m if m < GQ else 4
                        nc.scalar.activation(
                            raw[:, :], src_ps, AF.Identity,
                            bias=bias_sb[:, bcol:bcol + 1])
                        rot = pr.tile([128, 512], BF, tag="rot", bufs=2,
                                      name=f"rot_{t}_{m}")
                        nc.vector.tensor_copy(rot[0:64, :], raw[64:128, :])
                        nc.vector.tensor_copy(rot[64:128, :], raw[0:64, :])
                        t1 = pr.tile([128, 512], BF, tag="t1", bufs=2,
                                     name=f"t1_{t}_{m}")
                        nc.vector.tensor_mul(t1[:, :], rot[:, :],
                                             sin_sb[:, ts])
                        t2 = pr.tile([128, 512], BF, tag="t2", bufs=2,
                                     name=f"t2_{t}_{m}")
                        nc.vector.tensor_mul(t2[:, :], raw[:, :],
                                             cos_sb[:, ts])
                        dst = (qf_t[:, m, :] if m < GQ else kf[t][:, :])
                        nc.vector.tensor_add(dst, t1[:, :], t2[:, :])

                    # attention + o_proj for the PREVIOUS chunk run while
                    # this chunk's RoPE occupies the DVE
                    if t >= 1:
                        attention(t - 1, qf_tiles[t - 1])
                        oproj(t - 1, pwo, wo_sb)
                attention(TC - 1, qf_tiles[TC - 1])
                oproj(TC - 1, pwo, wo_sb)


def _build():
    global _NC
    if _NC is None:
        nc = bacc.Bacc("TRN2", target_bir_lowering=False, debug=False,
                       num_devices=8)
        _emit(nc)
        nc.compile()
        _NC = nc
    return _NC


def _prep_inputs(x, wq, bq, wk, bk, wv, bv, wo, bo, cos, sin):
    """Host-side shard + layout prep. Core c = (g, b): g = c % 4, b = c // 4."""
    import ml_dtypes
    bf16 = ml_dtypes.bfloat16
    inv_sqrt_d = 1.0 / math.sqrt(HD)
    f32 = np.float32
    cosT = np.ascontiguousarray(cos.T.astype(bf16))
    sinSf = sin.T.astype(f32).copy()
    sinSf[0:HD // 2] *= -1.0
    sinS = np.ascontiguousarray(sinSf.astype(bf16))

    xTb = [np.ascontiguousarray(x[b].T.astype(bf16)) for b in range(B)]

    in_maps = []
    for c in range(8):
        g, b = c % G, c // G
        wq_s = wq[QD * g:QD * (g + 1), :] * inv_sqrt_d
        bq_s = bq[QD * g:QD * (g + 1)] * inv_sqrt_d
        wk_s = wk[HD * g:HD * (g + 1), :]
        bk_s = bk[HD * g:HD * (g + 1)]
        wv_s = wv[HD * g:HD * (g + 1), :]
        bv_s = bv[HD * g:HD * (g + 1)]
        bias = np.zeros((128, 6), f32)
        bias[:, 0:4] = bq_s.reshape(GQ, HD).T
        bias[:, 4] = bk_s
        bias[:, 5] = bv_s
        in_maps.append({
            "xT": xTb[b],
            "wqT": np.ascontiguousarray(wq_s.T.astype(bf16)),
            "wkT": np.ascontiguousarray(wk_s.T.astype(bf16)),
            "wvT": np.ascontiguousarray(wv_s.T.astype(bf16)),
            "woT": np.ascontiguousarray(wo[:, QD * g:QD * (g + 1)].T
                                        .astype(bf16)),
            "cosT": cosT,
            "sinS": sinS,
            "bqkv": bias,
            "onesd": np.ones((128, 128), bf16),
        })
    return in_maps


def run(inputs, trace=False):
    """Returns (full_output, BassKernelResults)."""
    inputs = {k: np.asarray(v) for k, v in inputs.items()}
    nc = _build()
    in_maps = _prep_inputs(**inputs)
    res = run_bass_kernel_spmd(nc, in_maps, core_ids=list(range(8)),
                               trace=trace)
    bo = inputs["bo"].astype(np.float64)
    out = np.empty((B, S, H), np.float32)
    for b in range(B):
        acc = np.zeros((S, H), np.float64)
        for g in range(G):
            acc += res.results[G * b + g]["outp"].astype(np.float64)
        out[b] = (acc + bo).astype(np.float32)
    return out, res


def kernel(**inputs):
    return run(inputs, trace=False)[0]


# revision 9
# speedup vs baseline: 1.1521x; 1.1521x over previous
"""Trainium2 Bass kernel for GQA attention with RoPE (dense transformer).

Problem: B=2, S=2048, H=2048, 16 query heads / 4 KV heads, head_dim 128,
causal flash-style attention, fused QKV + o_proj.

Sharding (8 cores): (batch, head-group) grid. Core c handles batch c//4 and
head group c%4 (4 query heads + their shared KV head). o_proj is computed as
per-group partials reduced on host (tensor-parallel o_proj input split).

v2 vs baseline (336us):
  - bf16 activations/weights end-to-end (PE rate is identical to f32r, but
    DMA bytes and DVE element throughput both improve 2x; accuracy measured
    ~2e-3 rel, threshold 2e-2). PSUM accumulation stays fp32.
  - Softmax denominators: per (h,chunk) the exp tiles are accumulated over
    k-tiles on the DVE, then ONE ones-matmul computes the partition sum
    (was: one ones-matmul per k-tile; -144 PE matmuls).
  - Batched DMA: x loads as 1MB half-chunks, weights as whole tensors,
    outputs as one 0.5MB DMA per 128-token tile. Weights go on the ACT
    HWDGE queue, x/out on the SP queue.
  - o_proj is emitted interleaved (right after each q-chunk's attention),
    not as a tail phase, so its matmuls fill PE gaps left by exp pacing
    and the output DMA is spread across the kernel.

On-core layout: activations live as [feature, token] ("transposed") so the
feature contraction dims land on SBUF partitions for the PE array.
Causal masking: fully-masked k-tiles are skipped entirely; diagonal tiles
get a zero-fill triangle (affine_select on GpSimd) after exp.
"""
import math

import numpy as np

import concourse.bass as bass
import concourse.mybir as mybir
import concourse.tile as tile
from concourse import bacc
from concourse.bass_utils import run_bass_kernel_spmd
from concourse.masks import make_identity

B, S, H = 2, 2048, 2048
NH, KVH, HD = 16, 4, 128
G = 4                 # head groups (= KVH); grid = G x B = 8 cores
GQ = NH // KVH        # query heads per group
QD = GQ * HD          # per-core q dim (512)
KC = H // 128         # contraction chunks for projections (16)
TC = 4                # token chunks of 512
TT = S // 128         # 128-token tiles (16)

F32 = mybir.dt.float32
BF = mybir.dt.bfloat16
AF = mybir.ActivationFunctionType

_NC = None


def _emit(nc):
    # All big inputs are host-packed partition-major: row p is the full
    # contiguous per-partition payload, so every DMA is 128 descriptors of
    # >=4KB regardless of logical shape (HWDGE issue cost is ~5ns/descriptor).
    xP = nc.dram_tensor("xP", [128, KC * S], BF, kind="ExternalInput").ap()
    wqP = nc.dram_tensor("wqP", [128, KC * QD], BF, kind="ExternalInput").ap()
    wkP = nc.dram_tensor("wkP", [128, KC * HD], BF, kind="ExternalInput").ap()
    wvP = nc.dram_tensor("wvP", [128, KC * HD], BF, kind="ExternalInput").ap()
    woP = nc.dram_tensor("woP", [128, GQ * H], BF, kind="ExternalInput").ap()
    cosT = nc.dram_tensor("cosT", [HD, S], BF, kind="ExternalInput").ap()
    sinS = nc.dram_tensor("sinS", [HD, S], BF, kind="ExternalInput").ap()
    bqkv = nc.dram_tensor("bqkv", [128, 6], F32, kind="ExternalInput").ap()
    onesd = nc.dram_tensor("onesd", [128, 128], BF, kind="ExternalInput").ap()
    outp = nc.dram_tensor("outp", [S, H], BF, kind="ExternalOutput").ap()

    with tile.TileContext(nc) as tc:
        with (
            tc.tile_pool(name="persist", bufs=1) as pp,
            tc.tile_pool(name="qfp", bufs=2) as pqf,
            tc.tile_pool(name="cd", bufs=1) as pd,
            tc.tile_pool(name="expp", bufs=1) as pe,
            tc.tile_pool(name="psum8", bufs=1, space="PSUM") as ps8,
        ):
            # persistent per-chunk K/V (split per t-chunk to keep dep ranges
            # disjoint between the producing chunk and attention readers)
            kf = [pp.tile([128, 512], BF, name=f"kf{t}") for t in range(TC)]
            v_sb = [pp.tile([128, 4, HD], BF, name=f"vsb{t}")
                    for t in range(TC)]
            ofl = pd.tile([128, GQ, S], BF)       # normalized attn outT

            # ---- constants ----
            bias_sb = pp.tile([128, 6], F32)
            nc.gpsimd.dma_start(bias_sb[:, :], bqkv)
            ident = pp.tile([128, 128], BF)
            make_identity(nc, ident[:, :])
            ones_mat = pp.tile([128, 128], BF)
            nc.gpsimd.dma_start(ones_mat[:, :], onesd)

            def jspan(qc, j):
                if j < 4 * qc:
                    q0, n = 512 * qc, 512
                else:
                    q0 = 128 * j
                    n = 512 * (qc + 1) - q0
                return q0, n, q0 - 512 * qc

            def attention(qc, qf_t):
                """flash attention for q-chunk qc over k-tiles 0..4qc+3"""
                qs = slice(512 * qc, 512 * qc + 512)
                nj = 4 * qc + 4
                for h in range(GQ):
                    exs = pe.tile([128, 512], BF, tag="exs", bufs=3,
                                  name=f"exs_{h}_{qc}")
                    exts = []
                    for j in range(nj):
                        q0, n, off = jspan(qc, j)
                        ql = q0 - 512 * qc
                        ps = ps8.tile([128, 512], F32, tag=f"A{j % 4}",
                                      name=f"ps_{h}_{qc}_{j}")
                        nc.tensor.matmul(
                            ps[:, 0:n], kf[j // 4][:, 128 * (j % 4):
                                                   128 * (j % 4) + 128],
                            qf_t[:, h, ql:ql + n], start=True, stop=True)
                        ex = pe.tile([128, 512], BF, tag="E", bufs=20,
                                     name=f"ex_{h}_{qc}_{j}")
                        nc.scalar.activation(ex[:, 0:n], ps[:, 0:n], AF.Exp)
                        if j >= 4 * qc:
                            # zero the strictly-lower (q < k) triangle
                            nc.gpsimd.affine_select(
                                out=ex[:, 0:128], in_=ex[:, 0:128],
                                compare_op=mybir.AluOpType.is_ge, fill=0.0,
                                base=0, pattern=[[1, 128]],
                                channel_multiplier=-1)
                        if j == 0:
                            nc.vector.tensor_copy(exs[:, :], ex[:, :])
                        else:
                            nc.vector.tensor_add(exs[:, ql:ql + n],
                                                 exs[:, ql:ql + n],
                                                 ex[:, 0:n])
                        exts.append(ex)
                    p_sum = ps8.tile([128, 512], F32, tag="Bt", bufs=2,
                                     name=f"psum_{h}_{qc}")
                    nc.tensor.matmul(p_sum[:, :], ones_mat[:, :], exs[:, :],
                                     start=True, stop=True)
                    bc = pe.tile([128, 512], F32, tag="bc", bufs=2,
                                 name=f"bc_{h}_{qc}")
                    nc.vector.reciprocal_approx_fast(bc[:, :], p_sum[:, :])
                    p_o = ps8.tile([128, 512], F32, tag="Ct", bufs=2,
                                   name=f"po_{h}_{qc}")
                    for j in range(nj):
                        q0, n, off = jspan(qc, j)
                        nc.tensor.matmul(
                            p_o[:, off:off + n],
                            v_sb[j // 4][:, j % 4, :],
                            exts[j][:, 0:n], start=(j == 0), stop=(j == nj - 1))
                    nc.vector.tensor_mul(ofl[:, h, qs], p_o[:, :], bc[:, :])

            def oproj(qc, pwo, wo_sb):
                """o_proj partials for the 4 token-tiles of q-chunk qc."""
                for tt in range(4 * qc, 4 * qc + 4):
                    tsl = slice(128 * tt, 128 * tt + 128)
                    tags = ["Bt", "Bt", "Ct", "Ct"]
                    pfs = [ps8.tile([128, 512], F32, tag=tags[oc], bufs=2,
                                    name=f"pf_{tt}_{oc}")
                           for oc in range(4)]
                    for ic in range(GQ):
                        for oc in range(4):
                            osl = slice(512 * oc, 512 * oc + 512)
                            nc.tensor.matmul(
                                pfs[oc][:, :], ofl[:, ic, tsl],
                                wo_sb[:, ic, osl],
                                start=(ic == 0), stop=(ic == GQ - 1))
                    fo = pwo.tile([128, 4, 512], BF, tag="fo", bufs=3,
                                  name=f"fo_{tt}")
                    for oc in range(4):
                        # split psum evictions between ACT and DVE
                        if oc < 2:
                            nc.scalar.copy(fo[:, oc, :], pfs[oc][:, :])
                        else:
                            nc.vector.tensor_copy(fo[:, oc, :], pfs[oc][:, :])
                    nc.sync.dma_start(outp[tsl, :], fo[:, :, :])

            # ============ interleaved projections + attention =============
            qf_tiles = [None] * TC
            with (
                tc.tile_pool(name="projw", bufs=1) as pw,
                tc.tile_pool(name="rope", bufs=1) as pr,
                tc.tile_pool(name="wop", bufs=1) as pwo,
            ):
                x_sb = pw.tile([128, KC, S], BF)
                wq_sb = pw.tile([128, KC, QD], BF)
                wk_sb = pw.tile([128, KC, HD], BF)
                wv_sb = pw.tile([128, KC, HD], BF)
                cos_sb = pw.tile([128, S], BF)
                sin_sb = pw.tile([128, S], BF)
                wo_sb = pwo.tile([128, GQ, H], BF)

                # All input DMAs on the SP HWDGE queue, in first-needed
                # order, sized so the ko=0 accumulation starts after ~1MB.
                def ld_x(k0, k1):
                    nc.sync.dma_start(x_sb[:, k0:k1, :],
                                      xP[:, S * k0:S * k1])

                def ld_wq(k0, k1):
                    nc.sync.dma_start(wq_sb[:, k0:k1, :],
                                      wqP[:, QD * k0:QD * k1])

                ld_x(0, 1)
                ld_wq(0, 2)
                ld_x(1, 2)
                nc.sync.dma_start(wk_sb[:, :, :], wkP)
                nc.sync.dma_start(wv_sb[:, :, :], wvP)
                ld_x(2, 4)
                ld_wq(2, 4)
                ld_x(4, 8)
                ld_wq(4, 8)
                ld_x(8, 16)
                ld_wq(8, 16)
                nc.sync.dma_start(cos_sb[:, :], cosT)
                nc.sync.dma_start(sin_sb[:, :], sinS)
                nc.sync.dma_start(wo_sb[:, :, :], woP)

                for t in range(TC):
                    ts = slice(512 * t, 512 * t + 512)

                    pq = [ps8.tile([128, 512], F32, tag=f"A{m}",
                                   name=f"pq{m}_{t}")
                          for m in range(GQ)]
                    pk = ps8.tile([128, 512], F32, tag="Bt", bufs=2,
                                  name=f"pk_{t}")
                    pv = ps8.tile([128, 512], F32, tag="Bt", bufs=2,
                                  name=f"pv_{t}")
                    for ko in range(KC):
                        st = (ko == 0)
                        sp = (ko == KC - 1)
                        xc = x_sb[:, ko, ts]
                        for m in range(GQ):
                            nc.tensor.matmul(
                                pq[m][:, :],
                                wq_sb[:, ko, 128 * m:128 * m + 128],
                                xc, start=st, stop=sp)
                        nc.tensor.matmul(pk[:, :], wk_sb[:, ko, :],
                                         xc, start=st, stop=sp)
                        nc.tensor.matmul(pv[:, :], wv_sb[:, ko, :],
                                         xc, start=st, stop=sp)

                    # v: evict with bias, then transpose to natural layout
                    vT_t = pr.tile([128, 512], BF, tag="vT", bufs=2,
                                   name=f"vT_{t}")
                    nc.scalar.activation(vT_t[:, :], pv[:, :], AF.Identity,
                                         bias=bias_sb[:, 5:6])
                    for st4 in range(4):
                        ptr = ps8.tile([128, 128], BF, tag="Ct", bufs=2,
                                       name=f"ptr_{t}_{st4}")
                        nc.tensor.transpose(
                            ptr[:, :], vT_t[:, 128 * st4:128 * st4 + 128],
                            ident[:, :])
                        nc.scalar.copy(v_sb[t][:, st4, :], ptr[:, :])

                    # evict + bias; RoPE for q/k on DVE in [d, tok] layout
                    qf_t = pqf.tile([128, GQ, 512], BF, tag="qf",
                                    name=f"qf_{t}")
                    qf_tiles[t] = qf_t
                    for m in [GQ, 0, 1, 2, 3]:
                        raw = pr.tile([128, 512], BF, tag="raw", bufs=3,
                                      name=f"raw_{t}_{m}")
                        src_ps = pq[m][:, :] if m < GQ else pk[:, :]
                        bcol = m if m < GQ else 4
                        nc.scalar.activation(
                            raw[:, :], src_ps, AF.Identity,
                            bias=bias_sb[:, bcol:bcol + 1])
                        rot = pr.tile([128, 512], BF, tag="rot", bufs=2,
                                      name=f"rot_{t}_{m}")
                        nc.vector.tensor_copy(rot[0:64, :], raw[64:128, :])
                        nc.vector.tensor_copy(rot[64:128, :], raw[0:64, :])
                        t1 = pr.tile([128, 512], BF, tag="t1", bufs=2,
                                     name=f"t1_{t}_{m}")
                        nc.vector.tensor_mul(t1[:, :], rot[:, :],
                                             sin_sb[:, ts])
                        t2 = pr.tile([128, 512], BF, tag="t2", bufs=2,
                                     name=f"t2_{t}_{m}")
                        nc.vector.tensor_mul(t2[:, :], raw[:, :],
                                             cos_sb[:, ts])
                        dst = (qf_t[:, m, :] if m < GQ else kf[t][:, :])
                        nc.vector.tensor_add(dst, t1[:, :], t2[:, :])

                    # attention + o_proj for the PREVIOUS chunk run while
                    # this chunk's RoPE occupies the DVE
                    if t >= 1:
                        attention(t - 1, qf_tiles[t - 1])
                        oproj(t - 1, pwo, wo_sb)
                attention(TC - 1, qf_tiles[TC - 1])
                oproj(TC - 1, pwo, wo_sb)


def _build():
    global _NC
    if _NC is None:
        nc = bacc.Bacc("TRN2", target_bir_lowering=False, debug=False,
                       num_devices=8)
        _emit(nc)
        nc.compile()
        _NC = nc
    return _NC


def _prep_inputs(x, wq, bq, wk, bk, wv, bv, wo, bo, cos, sin):
    """Host-side shard + layout prep. Core c = (g, b): g = c % 4, b = c // 4."""
    import ml_dtypes
    bf16 = ml_dtypes.bfloat16
    inv_sqrt_d = 1.0 / math.sqrt(HD)
    f32 = np.float32
    cosT = np.ascontiguousarray(cos.T.astype(bf16))
    sinSf = sin.T.astype(f32).copy()
    sinSf[0:HD // 2] *= -1.0
    sinS = np.ascontiguousarray(sinSf.astype(bf16))

    def pack(mT):
        """[n*128, m] -> [128, n*m]: row p = concat_n mT[n*128+p, :]."""
        n = mT.shape[0] // 128
        return np.ascontiguousarray(
            mT.reshape(n, 128, mT.shape[1]).transpose(1, 0, 2)
            .reshape(128, n * mT.shape[1]).astype(bf16))

    xPb = [pack(x[b].T.astype(f32)) for b in range(B)]

    in_maps = []
    for c in range(8):
        g, b = c % G, c // G
        wq_s = wq[QD * g:QD * (g + 1), :] * inv_sqrt_d
        bq_s = bq[QD * g:QD * (g + 1)] * inv_sqrt_d
        wk_s = wk[HD * g:HD * (g + 1), :]
        bk_s = bk[HD * g:HD * (g + 1)]
        wv_s = wv[HD * g:HD * (g + 1), :]
        bv_s = bv[HD * g:HD * (g + 1)]
        bias = np.zeros((128, 6), f32)
        bias[:, 0:4] = bq_s.reshape(GQ, HD).T
        bias[:, 4] = bk_s
        bias[:, 5] = bv_s
        in_maps.append({
            "xP": xPb[b],
            "wqP": pack(wq_s.T),
            "wkP": pack(wk_s.T),
            "wvP": pack(wv_s.T),
            "woP": pack(wo[:, QD * g:QD * (g + 1)].T),
            "cosT": cosT,
            "sinS": sinS,
            "bqkv": bias,
            "onesd": np.ones((128, 128), bf16),
        })
    return in_maps


def run(inputs, trace=False):
    """Returns (full_output, BassKernelResults)."""
    inputs = {k: np.asarray(v) for k, v in inputs.items()}
    nc = _build()
    in_maps = _prep_inputs(**inputs)
    res = run_bass_kernel_spmd(nc, in_maps, core_ids=list(range(8)),
                               trace=trace)
    bo = inputs["bo"].astype(np.float64)
    out = np.empty((B, S, H), np.float32)
    for b in range(B):
        acc = np.zeros((S, H), np.float64)
        for g in range(G):
            acc += res.results[G * b + g]["outp"].astype(np.float64)
        out[b] = (acc + bo).astype(np.float32)
    return out, res


def kernel(**inputs):
    return run(inputs, trace=False)[0]
